# revision 1
# baseline (speedup 1.0000x reference)
"""Distributed Trainium2 kernel for nn_AttentionBlock (B=2, N=2048, D=1024, H=16).

Sharding: the 32 (batch, head) attention units are split 4-per-core across the
8 NeuronCores (2 heads x 2 batches each).  Each core computes the QKV
projection for its 2 heads over the full sequence, full attention for its 4
(b, h) units, and a partial output projection (contraction over its 128 head
dims).  The 8 partial [4096, 1024] products are summed on the host (the
all-reduce of tensor-parallel out-proj), so the device graph needs no
collectives.

Layouts (per core):
  xt    [128, 8, 4096]  x^T as (d_inner=128, d_outer=8, n)      bf16
  wqkv  [128, 8, 384]   W_qkv_core^T as (d_inner, d_outer, e)   bf16
  bqkv  [128, 3]        bias per e-tile (q-part pre-scaled by 1/8)
  wo    [128, 1024]     W_o[:, core cols]^T  (e_local, d_out)   f32
  y     [4096, 1024]    partial out-proj                        f32

Pipeline: QKV (d on partitions) -> PE-transpose V to (k, hd) -> scores^T
(k, q) in f32r -> exp on ACT to bf16 -> PV matmul with a ones-column to get
row sums -> PE-transpose (q on partitions) -> normalize by 1/sumexp ->
PE-transpose back (e on partitions) -> out-proj in f32r -> DMA PSUM->DRAM.
"""

import os
import numpy as np
import ml_dtypes

import concourse.bass as bass
import concourse.tile as tile
from concourse import bacc, mybir
from concourse.bass_utils import run_bass_kernel_spmd
from concourse.masks import make_identity

B, N, D = 2, 2048, 1024
H, HD = 16, 64
NCORES = 8
BN = B * N  # 4096
HPC = H // NCORES  # heads per core = 2

F32 = mybir.dt.float32
F32R = mybir.dt.float32r
BF16 = mybir.dt.bfloat16
AF = mybir.ActivationFunctionType


def build_nc():
    nc = bacc.Bacc(
        "TRN2", target_bir_lowering=False, debug=False, num_devices=NCORES
    )
    xt = nc.dram_tensor("xt", [128, 8, BN], BF16, kind="ExternalInput").ap()
    wqkv = nc.dram_tensor("wqkv", [128, 8, 384], BF16, kind="ExternalInput").ap()
    bqkv = nc.dram_tensor("bqkv", [128, 3], F32, kind="ExternalInput").ap()
    wo = nc.dram_tensor("wo", [128, 1024], BF16, kind="ExternalInput").ap()
    y = nc.dram_tensor("y", [BN, D], F32, kind="ExternalOutput").ap()

    with tile.TileContext(nc) as tc:
        _body(nc, tc, xt, wqkv, bqkv, wo, y)
    nc.compile()
    return nc


def _body(nc, tc, xt, wqkv, bqkv, wo, y):
    from contextlib import ExitStack

    with ExitStack() as ctx:
        const_pool = ctx.enter_context(tc.tile_pool(name="const", bufs=1))
        xpool = ctx.enter_context(tc.tile_pool(name="xp", bufs=2))
        qkv_pool = ctx.enter_context(tc.tile_pool(name="qkv", bufs=1))
        vaug_pool = ctx.enter_context(tc.tile_pool(name="vaug", bufs=1))
        exps_pool = ctx.enter_context(tc.tile_pool(name="exps", bufs=2))
        ao_pool = ctx.enter_context(tc.tile_pool(name="ao", bufs=1))
        aot_pool = ctx.enter_context(tc.tile_pool(name="aot", bufs=1))
        u_pool = ctx.enter_context(tc.tile_pool(name="u", bufs=3))
        y_pool = ctx.enter_context(tc.tile_pool(name="ysb", bufs=3))
        rec_pool = ctx.enter_context(tc.tile_pool(name="rec", bufs=4))
        ps_big = ctx.enter_context(tc.tile_pool(name="psb", bufs=4, space="PSUM"))
        ps_pv = ctx.enter_context(tc.tile_pool(name="pspv", bufs=2, space="PSUM"))
        ps_t = ctx.enter_context(tc.tile_pool(name="pst", bufs=2, space="PSUM"))

        w_sb = const_pool.tile([128, 8, 384], BF16)
        nc.sync.dma_start(out=w_sb[:], in_=wqkv[:])
        bias_sb = const_pool.tile([128, 3], F32)
        nc.sync.dma_start(out=bias_sb[:], in_=bqkv[:])
        wo_sb = const_pool.tile([128, 1024], BF16)
        nc.sync.dma_start(out=wo_sb[:], in_=wo[:])
        ident = const_pool.tile([128, 128], F32)
        make_identity(nc, ident[:])

        qT = qkv_pool.tile([128, BN], BF16, tag="qT")
        kT = qkv_pool.tile([128, BN], BF16, tag="kT")
        vT = qkv_pool.tile([128, BN], F32, tag="vT")

        # ---- Phase 1: QKV projection (output transposed: e on partitions) ----
        for ncid in range(8):
            nsl = slice(ncid * 512, (ncid + 1) * 512)
            xtile = xpool.tile([128, 8, 512], BF16, tag="xt")
            nc.sync.dma_start(out=xtile[:], in_=xt[:, :, nsl])
            pss3 = [ps_big.tile([128, 512], F32, tag="big", name=f"qkvps{ncid}_{_i}") for _i in range(3)]
            for dc in range(8):
                for et in range(3):
                    nc.tensor.matmul(
                        pss3[et][:],
                        w_sb[:, dc, et * 128 : (et + 1) * 128],
                        xtile[:, dc, :],
                        start=(dc == 0),
                        stop=(dc == 7),
                    )
            for et, dst, scale in ((0, qT, 0.125), (1, kT, 1.0), (2, vT, 1.0)):
                nc.vector.tensor_scalar(
                    out=dst[:, nsl], in0=pss3[et][:],
                    scalar1=scale, scalar2=bias_sb[:, et : et + 1],
                    op0=mybir.AluOpType.mult, op1=mybir.AluOpType.add,
                )

        # ---- Phase 2: transpose V to (k on partitions, hd) + ones column ----
        vaug = vaug_pool.tile([128, 4, 16, 65], BF16)
        nc.vector.memset(vaug[:, :, :, 64:65], 1.0)
        for p in range(4):
            b, h = p // 2, p % 2
            hs = slice(h * 64, (h + 1) * 64)
            for kb in range(16):
                ksl = slice(b * 2048 + kb * 128, b * 2048 + (kb + 1) * 128)
                pst = ps_t.tile([128, 64], F32, tag="pst")
                nc.tensor.transpose(pst[:], vT[hs, ksl], ident[hs, hs])
                nc.vector.tensor_copy(out=vaug[:, p, kb, 0:64], in_=pst[:])

        # ---- Phase 3: attention per (b, h) unit ----
        ao = ao_pool.tile([128, 32, 128], F32)  # (q_part, nblk, e_local)
        for p in range(4):
            b, h = p // 2, p % 2
            hs = slice(h * 64, (h + 1) * 64)
            for qc in range(4):
                qsl = slice(b * 2048 + qc * 512, b * 2048 + (qc + 1) * 512)
                exps = exps_pool.tile([128, 16, 512], BF16, tag="exps")
                for kb in range(16):
                    ksl = slice(b * 2048 + kb * 128, b * 2048 + (kb + 1) * 128)
                    pss = ps_big.tile([128, 512], F32, tag="big")
                    nc.tensor.matmul(
                        pss[:],
                        kT[hs, ksl],
                        qT[hs, qsl],
                        start=True,
                        stop=True,
                    )
                    nc.scalar.activation(exps[:, kb, :], pss[:], AF.Exp)
                pv2 = [ps_pv.tile([65, 512], F32, tag="pv", name=f"pv{p}_{qc}_{_i}") for _i in range(2)]
                for kb in range(16):
                    nc.tensor.matmul(
                        pv2[kb % 2][:],
                        vaug[:, p, kb, :],
                        exps[:, kb, :],
                        start=(kb < 2),
                        stop=(kb >= 14),
                    )
                u = u_pool.tile([65, 512], F32, tag="u")
                nc.vector.tensor_copy(out=u[:], in_=pv2[0][:])
                nc.vector.tensor_add(u[:], u[:], pv2[1][:])
                for qb in range(4):
                    nblk = b * 16 + qc * 4 + qb
                    pst2 = ps_t.tile([128, 65], F32, tag="pst")
                    nc.tensor.transpose(
                        pst2[:], u[:, qb * 128 : (qb + 1) * 128], ident[0:65, 0:65]
                    )
                    rec = rec_pool.tile([128, 1], F32, tag="rec")
                    nc.vector.reciprocal(out=rec[:], in_=pst2[:, 64:65])
                    nc.vector.tensor_scalar_mul(
                        ao[:, nblk, hs], pst2[:, 0:64], rec[:]
                    )

        # ---- Phase 4/5: transpose back (e on partitions) + partial out-proj ----
        aoT = aot_pool.tile([128, BN], BF16)
        for nb in range(32):
            nbl = slice(nb * 128, (nb + 1) * 128)
            pst3 = ps_t.tile([128, 128], F32, tag="pst")
            nc.tensor.transpose(pst3[:], ao[:, nb, :], ident[:])
            nc.vector.tensor_copy(out=aoT[:, nbl], in_=pst3[:])
            for dc in range(2):
                dsl = slice(dc * 512, (dc + 1) * 512)
                psy = ps_big.tile([128, 512], F32, tag="big")
                nc.tensor.matmul(
                    psy[:],
                    aoT[:, nbl],
                    wo_sb[:, dsl],
                    start=True,
                    stop=True,
                )
                ysb = y_pool.tile([128, 512], F32, tag="ysb")
                nc.vector.tensor_copy(out=ysb[:], in_=psy[:])
                nc.sync.dma_start(out=y[nbl, dsl], in_=ysb[:])


def make_in_maps(x, W_qkv, b_qkv, W_o):
    x2 = np.asarray(x, dtype=np.float32).reshape(BN, D)
    xt_np = np.ascontiguousarray(
        x2.T.reshape(8, 128, BN).transpose(1, 0, 2)
    ).astype(ml_dtypes.bfloat16)

    in_maps = []
    for c in range(NCORES):
        rq = slice(2 * c * 64, 2 * c * 64 + 128)
        rk = slice(D + 128 * c, D + 128 * (c + 1))
        rv = slice(2 * D + 128 * c, 2 * D + 128 * (c + 1))
        Wc = np.concatenate([W_qkv[rq], W_qkv[rk], W_qkv[rv]], axis=0)  # [384, 1024]
        wc_np = np.ascontiguousarray(
            Wc.T.reshape(8, 128, 384).transpose(1, 0, 2)
        ).astype(ml_dtypes.bfloat16)
        bc = np.concatenate(
            [b_qkv[rq] * 0.125, b_qkv[rk], b_qkv[rv]]
        ).astype(np.float32)
        bc_np = np.ascontiguousarray(bc.reshape(3, 128).T)  # [128, 3]
        wo_np = np.ascontiguousarray(
            W_o[:, 128 * c : 128 * (c + 1)].T
        ).astype(ml_dtypes.bfloat16)  # [128, 1024]
        in_maps.append({"xt": xt_np, "wqkv": wc_np, "bqkv": bc_np, "wo": wo_np})
    return in_maps


_NC_CACHE = {}


def get_nc():
    if "nc" not in _NC_CACHE:
        _NC_CACHE["nc"] = build_nc()
    return _NC_CACHE["nc"]


def run(inputs, trace=False, **kw):
    nc = get_nc()
    in_maps = make_in_maps(
        np.asarray(inputs["x"]),
        np.asarray(inputs["W_qkv"]),
        np.asarray(inputs["b_qkv"]),
        np.asarray(inputs["W_o"]),
    )
    res = run_bass_kernel_spmd(
        nc, in_maps, core_ids=list(range(NCORES)), trace=trace, **kw
    )
    parts = [np.asarray(m["y"], dtype=np.float32) for m in res.results]
    y = parts[0]
    for pt in parts[1:]:
        y = y + pt
    y = y + np.asarray(inputs["b_o"], dtype=np.float32)[None, :]
    return y.reshape(B, N, D), res


def kernel(x, W_qkv, b_qkv, W_o, b_o):
    y, _ = run({"x": x, "W_qkv": W_qkv, "b_qkv": b_qkv, "W_o": W_o, "b_o": b_o})
    return y



# revision 4
# speedup vs baseline: 1.8597x; 1.8597x over previous
"""Distributed Trainium2 kernel for nn_AttentionBlock (B=2, N=2048, D=1024, H=16).

Sharding v2: 1 batch x 4 heads per core (batch = core//4, head group = core%4).
Each core computes QKV for its 4 heads over its batch's 2048 tokens, full
attention for those 4 (b, h) units, and a partial out-projection contracting
its 256 local attention-out dims.  The 4 partial [2048, 1024] products per
batch are summed on the host (tensor-parallel all-reduce), plus an exact
host-side bias correction y += W_o @ b_v + b_o (k-bias cancels in softmax,
q-bias is applied on-device per-partition).

Per-core layouts:
  xt    [4, 128, 8, 512]  x[b]^T chunked by n-block (d_inner, d_outer, n)  bf16
  wqkv  [128, 8, 768]     W_qkv rows for 4 heads, (d_inner, d_outer, e)    bf16
                          e-order: q0 q1 k0 k1 v0 v1 (128 each)
  bqkv  [128, 4]          bias for q0 q1 (pre-scaled 1/8) k0 k1            f32
  wo    [128, 2, 1024]    W_o[:, local cols]^T (e_inner, e_outer, d_out)   bf16
  y     [2048, 1024]      partial out-proj                                 bf16

Pipeline: QKV q,k (e on partitions) -> V via flipped matmul directly into
(k, hd) layout with a ones column -> per (qc, head): scores (k, q) in pairs
of 512-col PSUM banks -> exp on ACT over 1024 cols -> PV with ones column
giving row sums -> bf16 PE-transpose (q on partitions) -> normalize by
1/sumexp -> bf16 PE-transpose back (e on partitions) -> out-proj -> y DMA,
interleaved per qc so the tail is one tile deep.
"""

import numpy as np
import ml_dtypes

import concourse.bass as bass
import concourse.tile as tile
from concourse import bacc, mybir
from concourse.bass_utils import run_bass_kernel_spmd
from concourse.masks import make_identity

B, N, D = 2, 2048, 1024
H, HD = 16, 64
NCORES = 8
HPC = 4  # heads per core

F32 = mybir.dt.float32
BF16 = mybir.dt.bfloat16
AF = mybir.ActivationFunctionType
MUL = mybir.AluOpType.mult
ADD = mybir.AluOpType.add


def build_nc():
    nc = bacc.Bacc(
        "TRN2", target_bir_lowering=False, debug=False, num_devices=NCORES
    )
    xt = nc.dram_tensor("xt", [4, 128, 8, 512], BF16, kind="ExternalInput").ap()
    wqkv = nc.dram_tensor("wqkv", [128, 8, 768], BF16, kind="ExternalInput").ap()
    bqkv = nc.dram_tensor("bqkv", [128, 4], F32, kind="ExternalInput").ap()
    wo = nc.dram_tensor("wo", [128, 2, 1024], BF16, kind="ExternalInput").ap()
    y = nc.dram_tensor("y", [N, D], BF16, kind="ExternalOutput").ap()

    with tile.TileContext(nc) as tc:
        _body(nc, tc, xt, wqkv, bqkv, wo, y)
    nc.compile()
    return nc


def _body(nc, tc, xt, wqkv, bqkv, wo, y):
    from contextlib import ExitStack

    with ExitStack() as ctx:
        const_pool = ctx.enter_context(tc.tile_pool(name="const", bufs=1))
        qkv_pool = ctx.enter_context(tc.tile_pool(name="qkv", bufs=1))
        exps_pool = ctx.enter_context(tc.tile_pool(name="exps", bufs=2))
        u_pool = ctx.enter_context(tc.tile_pool(name="u", bufs=2))
        ao_pool = ctx.enter_context(tc.tile_pool(name="ao", bufs=2))
        aot_pool = ctx.enter_context(tc.tile_pool(name="aot", bufs=2))
        rec_pool = ctx.enter_context(tc.tile_pool(name="rec", bufs=4))
        y_pool = ctx.enter_context(tc.tile_pool(name="ysb", bufs=3))
        ps_big = ctx.enter_context(tc.tile_pool(name="psb", bufs=2, space="PSUM"))
        ps_sc = ctx.enter_context(tc.tile_pool(name="pssc", bufs=2, space="PSUM"))
        ps_u = ctx.enter_context(tc.tile_pool(name="psu", bufs=1, space="PSUM"))
        ps_t = ctx.enter_context(tc.tile_pool(name="pst", bufs=1, space="PSUM"))

        w_sb = const_pool.tile([128, 8, 768], BF16)
        nc.sync.dma_start(out=w_sb[:, :, 0:512], in_=wqkv[:, :, 0:512])
        nc.sync.dma_start(out=w_sb[:, :, 512:768], in_=wqkv[:, :, 512:768])
        bias_sb = const_pool.tile([128, 4], F32)
        nc.sync.dma_start(out=bias_sb[:], in_=bqkv[:])
        wo_sb = const_pool.tile([128, 2, 1024], BF16)
        nc.sync.dma_start(out=wo_sb[:], in_=wo[:])
        xt_sb = const_pool.tile([128, 8, 4, 512], BF16)
        for i in range(4):
            nc.sync.dma_start(out=xt_sb[:, :, i, :], in_=xt[i])
        ident = const_pool.tile([128, 128], BF16)
        make_identity(nc, ident[:])

        qT = qkv_pool.tile([128, 2, N], BF16, tag="qT")
        kT = qkv_pool.tile([128, 2, N], BF16, tag="kT")
        vaug = qkv_pool.tile([128, HPC, 16, 65], BF16, tag="vaug")
        nc.vector.memset(vaug[:, :, :, 64:65], 1.0)

        # ---- Phase 1a: Q, K projection (e on partitions) ----
        for i in range(4):
            nsl = slice(i * 512, (i + 1) * 512)
            for et in range(4):  # q0 q1 k0 k1
                ps = ps_big.tile([128, 512], F32, tag="big", name=f"qk_ps{i}_{et}")
                for dc in range(8):
                    nc.tensor.matmul(
                        ps[:],
                        w_sb[:, dc, et * 128 : (et + 1) * 128],
                        xt_sb[:, dc, i, :],
                        start=(dc == 0),
                        stop=(dc == 7),
                    )
                dst = qT if et < 2 else kT
                nc.vector.tensor_scalar(
                    out=dst[:, et % 2, nsl], in0=ps[:],
                    scalar1=(0.125 if et < 2 else 1.0),
                    scalar2=bias_sb[:, et : et + 1],
                    op0=MUL, op1=ADD,
                )

        # ---- Phase 1b: V projection, flipped (n on partitions) ----
        for nb in range(16):
            i, sub = nb // 4, nb % 4
            psv = ps_big.tile([128, 512], F32, tag="big", name=f"v_ps{nb}")
            for dc in range(8):
                nc.tensor.matmul(
                    psv[:, 0:256],
                    xt_sb[:, dc, i, sub * 128 : (sub + 1) * 128],
                    w_sb[:, dc, 512:768],
                    start=(dc == 0),
                    stop=(dc == 7),
                )
            nc.vector.tensor_copy(
                out=vaug[:, :, nb, 0:64],
                in_=psv[:, 0:256].rearrange("p (h d) -> p h d", h=HPC),
            )

        # ---- Phase 2: attention + out-proj, per 512-token q-chunk ----
        for i in range(4):
            qsl = slice(i * 512, (i + 1) * 512)
            aoq = ao_pool.tile([128, 4, 256], BF16, tag="aoq")  # (q, qb, e_loc)
            for h in range(HPC):
                hs = slice((h % 2) * 64, (h % 2) * 64 + 64)
                ho = h // 2
                exps = exps_pool.tile([128, 16, 512], BF16, tag="exps")
                for kp in range(8):
                    ps2 = ps_sc.tile([128, 2, 512], F32, tag="sc")
                    for j in range(2):
                        kb = kp * 2 + j
                        nc.tensor.matmul(
                            ps2[:, j, :],
                            kT[hs, ho, kb * 128 : (kb + 1) * 128],
                            qT[hs, ho, qsl],
                            start=True,
                            stop=True,
                        )
                    nc.scalar.activation(
                        exps[:, kp * 2 : kp * 2 + 2, :], ps2[:], AF.Exp
                    )
                psu = ps_u.tile([65, 512], F32, tag="psu")
                for kb in range(16):
                    nc.tensor.matmul(
                        psu[:],
                        vaug[:, h, kb, :],
                        exps[:, kb, :],
                        start=(kb == 0),
                        stop=(kb == 15),
                    )
                ub = u_pool.tile([65, 512], BF16, tag="ub")
                nc.vector.tensor_copy(out=ub[:], in_=psu[:])
                pst = ps_t.tile([128, 8, 128], BF16, tag="pst")
                for qb in range(4):
                    nc.tensor.transpose(
                        pst[:, qb, 0:65], ub[:, qb * 128 : (qb + 1) * 128],
                        ident[0:65, 0:65],
                    )
                rec = rec_pool.tile([128, 4], F32, tag="rec")
                nc.vector.reciprocal(out=rec[:], in_=pst[:, 0:4, 64])
                for qb in range(4):
                    nc.vector.tensor_scalar_mul(
                        aoq[:, qb, h * 64 : (h + 1) * 64],
                        pst[:, qb, 0:64],
                        rec[:, qb : qb + 1],
                    )
            # transpose back (e on partitions) and out-project this q-chunk
            aoT = aot_pool.tile([128, 2, 512], BF16, tag="aoT")
            pstT = ps_t.tile([128, 8, 128], BF16, tag="pst")
            for qb in range(4):
                for eo in range(2):
                    nc.tensor.transpose(
                        pstT[:, eo * 4 + qb, :],
                        aoq[:, qb, eo * 128 : (eo + 1) * 128],
                        ident[:],
                    )
            for eo in range(2):
                nc.vector.tensor_copy(
                    out=aoT[:, eo, :].rearrange("p (qb q) -> p qb q", qb=4),
                    in_=pstT[:, eo * 4 : eo * 4 + 4, :],
                )
            for qb in range(4):
                for dc in range(2):
                    dsl = slice(dc * 512, (dc + 1) * 512)
                    psy = ps_big.tile([128, 512], F32, tag="big", name=f"y_ps{i}_{qb}_{dc}")
                    nc.tensor.matmul(
                        psy[:],
                        aoT[:, 0, qb * 128 : (qb + 1) * 128],
                        wo_sb[:, 0, dsl],
                        start=True,
                        stop=False,
                    )
                    nc.tensor.matmul(
                        psy[:],
                        aoT[:, 1, qb * 128 : (qb + 1) * 128],
                        wo_sb[:, 1, dsl],
                        start=False,
                        stop=True,
                    )
                    ysb = y_pool.tile([128, 512], BF16, tag="ysb")
                    nc.vector.tensor_copy(out=ysb[:], in_=psy[:])
                    nc.sync.dma_start(
                        out=y[i * 512 + qb * 128 : i * 512 + (qb + 1) * 128, dsl],
                        in_=ysb[:],
                    )


def make_in_maps(x, W_qkv, b_qkv, W_o):
    bf = ml_dtypes.bfloat16
    in_maps = []
    xt_b = []
    for b in range(B):
        x2 = np.asarray(x[b], dtype=np.float32)  # [2048, 1024]
        # [n, d] -> [d, n] -> [8, 128, n] -> [128, 8, n] -> chunk n by 512
        xt = x2.T.reshape(8, 128, N).transpose(1, 0, 2)
        xt = np.ascontiguousarray(
            xt.reshape(128, 8, 4, 512).transpose(2, 0, 1, 3)
        ).astype(bf)
        xt_b.append(xt)
    for c in range(NCORES):
        b, hg = c // 4, c % 4
        sl = slice(hg * 256, hg * 256 + 256)
        Wc = np.concatenate(
            [W_qkv[0:D][sl], W_qkv[D : 2 * D][sl], W_qkv[2 * D : 3 * D][sl]],
            axis=0,
        )  # [768, 1024]
        wc = np.ascontiguousarray(
            Wc.T.reshape(8, 128, 768).transpose(1, 0, 2)
        ).astype(bf)
        bc = np.concatenate(
            [b_qkv[0:D][sl] * 0.125, b_qkv[D : 2 * D][sl]]
        ).astype(np.float32)  # [512]
        bc = np.ascontiguousarray(bc.reshape(4, 128).T)  # [128, 4]
        woc = np.ascontiguousarray(
            W_o[:, sl].T.reshape(2, 128, 1024).transpose(1, 0, 2)
        ).astype(bf)  # [128, 2, 1024]
        in_maps.append({"xt": xt_b[b], "wqkv": wc, "bqkv": bc, "wo": woc})
    return in_maps


_NC_CACHE = {}


def get_nc():
    if "nc" not in _NC_CACHE:
        _NC_CACHE["nc"] = build_nc()
    return _NC_CACHE["nc"]


def run(inputs, trace=False, **kw):
    nc = get_nc()
    x = np.asarray(inputs["x"])
    W_qkv = np.asarray(inputs["W_qkv"], dtype=np.float32)
    b_qkv = np.asarray(inputs["b_qkv"], dtype=np.float32)
    W_o = np.asarray(inputs["W_o"], dtype=np.float32)
    b_o = np.asarray(inputs["b_o"], dtype=np.float32)
    in_maps = make_in_maps(x, W_qkv, b_qkv, W_o)
    res = run_bass_kernel_spmd(
        nc, in_maps, core_ids=list(range(NCORES)), trace=trace, **kw
    )
    parts = [np.asarray(m["y"], dtype=np.float32) for m in res.results]
    yb = []
    for b in range(B):
        yy = parts[4 * b]
        for g in range(1, 4):
            yy = yy + parts[4 * b + g]
        yb.append(yy)
    yout = np.stack(yb, axis=0)  # [B, N, D]
    # exact bias correction: v-bias flows through attention (rows sum to 1)
    # into out-proj; k-bias cancels in softmax; q-bias applied on device.
    corr = W_o @ b_qkv[2 * D : 3 * D] + b_o
    yout = yout + corr[None, None, :]
    return yout, res


def kernel(x, W_qkv, b_qkv, W_o, b_o):
    y, _ = run({"x": x, "W_qkv": W_qkv, "b_qkv": b_qkv, "W_o": W_o, "b_o": b_o})
    return y


# revision 6
# speedup vs baseline: 1.8863x; 1.0143x over previous
"""Distributed Trainium2 kernel for nn_AttentionBlock (B=2, N=2048, D=1024, H=16).

Sharding v2: 1 batch x 4 heads per core (batch = core//4, head group = core%4).
Each core computes QKV for its 4 heads over its batch's 2048 tokens, full
attention for those 4 (b, h) units, and a partial out-projection contracting
its 256 local attention-out dims.  The 4 partial [2048, 1024] products per
batch are summed on the host (tensor-parallel all-reduce), plus an exact
host-side bias correction y += W_o @ b_v + b_o (k-bias cancels in softmax,
q-bias is applied on-device per-partition).

Per-core layouts:
  xt    [4, 128, 8, 512]  x[b]^T chunked by n-block (d_inner, d_outer, n)  bf16
  wqkv  [128, 8, 768]     W_qkv rows for 4 heads, (d_inner, d_outer, e)    bf16
                          e-order: q0 q1 k0 k1 v0 v1 (128 each)
  bqkv  [128, 4]          bias for q0 q1 (pre-scaled 1/8) k0 k1            f32
  wo    [128, 2, 1024]    W_o[:, local cols]^T (e_inner, e_outer, d_out)   bf16
  y     [2048, 1024]      partial out-proj                                 bf16

Pipeline: QKV q,k (e on partitions) -> V via flipped matmul directly into
(k, hd) layout with a ones column -> per (qc, head): scores (k, q) in pairs
of 512-col PSUM banks -> exp on ACT over 1024 cols -> PV with ones column
giving row sums -> bf16 PE-transpose (q on partitions) -> normalize by
1/sumexp -> bf16 PE-transpose back (e on partitions) -> out-proj -> y DMA,
interleaved per qc so the tail is one tile deep.
"""

import numpy as np
import ml_dtypes

import concourse.bass as bass
import concourse.tile as tile
from concourse import bacc, mybir
from concourse.bass_utils import run_bass_kernel_spmd
from concourse.masks import make_identity

B, N, D = 2, 2048, 1024
H, HD = 16, 64
NCORES = 8
HPC = 4  # heads per core

F32 = mybir.dt.float32
BF16 = mybir.dt.bfloat16
AF = mybir.ActivationFunctionType
MUL = mybir.AluOpType.mult
ADD = mybir.AluOpType.add


def build_nc():
    nc = bacc.Bacc(
        "TRN2", target_bir_lowering=False, debug=False, num_devices=NCORES
    )
    xt = nc.dram_tensor("xt", [4, 128, 8, 512], BF16, kind="ExternalInput").ap()
    wqkv = nc.dram_tensor("wqkv", [128, 8, 768], BF16, kind="ExternalInput").ap()
    bqkv = nc.dram_tensor("bqkv", [128, 4], F32, kind="ExternalInput").ap()
    wo = nc.dram_tensor("wo", [128, 2, 1024], BF16, kind="ExternalInput").ap()
    y = nc.dram_tensor("y", [N, D], BF16, kind="ExternalOutput").ap()

    with tile.TileContext(nc) as tc:
        _body(nc, tc, xt, wqkv, bqkv, wo, y)
    nc.compile()
    return nc


def _body(nc, tc, xt, wqkv, bqkv, wo, y):
    from contextlib import ExitStack

    with ExitStack() as ctx:
        const_pool = ctx.enter_context(tc.tile_pool(name="const", bufs=1))
        qkv_pool = ctx.enter_context(tc.tile_pool(name="qkv", bufs=1))
        exps_pool = ctx.enter_context(tc.tile_pool(name="exps", bufs=2))
        u_pool = ctx.enter_context(tc.tile_pool(name="u", bufs=2))
        ao_pool = ctx.enter_context(tc.tile_pool(name="ao", bufs=2))
        aot_pool = ctx.enter_context(tc.tile_pool(name="aot", bufs=2))
        rec_pool = ctx.enter_context(tc.tile_pool(name="rec", bufs=4))
        y_pool = ctx.enter_context(tc.tile_pool(name="ysb", bufs=3))
        ps_big = ctx.enter_context(tc.tile_pool(name="psb", bufs=2, space="PSUM"))
        ps_sc = ctx.enter_context(tc.tile_pool(name="pssc", bufs=2, space="PSUM"))
        ps_u = ctx.enter_context(tc.tile_pool(name="psu", bufs=1, space="PSUM"))
        ps_t = ctx.enter_context(tc.tile_pool(name="pst", bufs=1, space="PSUM"))

        # DMA order follows the critical path: bias + q/k weights + first x
        # chunk gate the first QKV matmul; v weights and W_o are needed later.
        bias_sb = const_pool.tile([128, 4], F32)
        nc.sync.dma_start(out=bias_sb[:], in_=bqkv[:])
        w_sb = const_pool.tile([128, 8, 768], BF16)
        nc.sync.dma_start(out=w_sb[:, :, 0:512], in_=wqkv[:, :, 0:512])
        xt_sb = const_pool.tile([128, 8, 4, 512], BF16)
        for i in range(4):
            nc.sync.dma_start(out=xt_sb[:, :, i, :], in_=xt[i])
        nc.sync.dma_start(out=w_sb[:, :, 512:768], in_=wqkv[:, :, 512:768])
        wo_sb = const_pool.tile([128, 2, 1024], BF16)
        nc.sync.dma_start(out=wo_sb[:], in_=wo[:])
        ident = const_pool.tile([128, 128], BF16)
        make_identity(nc, ident[:])

        qT = qkv_pool.tile([128, 2, N], BF16, tag="qT")
        kT = qkv_pool.tile([128, 2, N], BF16, tag="kT")
        vaug = qkv_pool.tile([128, HPC, 16, 65], BF16, tag="vaug")
        nc.vector.memset(vaug[:, :, :, 64:65], 1.0)

        # ---- Phase 1a: Q, K projection (e on partitions) ----
        for i in range(4):
            nsl = slice(i * 512, (i + 1) * 512)
            for et in range(4):  # q0 q1 k0 k1
                ps = ps_big.tile([128, 512], F32, tag="big", name=f"qk_ps{i}_{et}")
                for dc in range(8):
                    nc.tensor.matmul(
                        ps[:],
                        w_sb[:, dc, et * 128 : (et + 1) * 128],
                        xt_sb[:, dc, i, :],
                        start=(dc == 0),
                        stop=(dc == 7),
                    )
                dst = qT if et < 2 else kT
                nc.vector.tensor_scalar(
                    out=dst[:, et % 2, nsl], in0=ps[:],
                    scalar1=(0.125 if et < 2 else 1.0),
                    scalar2=bias_sb[:, et : et + 1],
                    op0=MUL, op1=ADD,
                )

        # ---- Phase 1b: V projection, flipped (n on partitions) ----
        for nb in range(16):
            i, sub = nb // 4, nb % 4
            psv = ps_big.tile([128, 512], F32, tag="big", name=f"v_ps{nb}")
            for dc in range(8):
                nc.tensor.matmul(
                    psv[:, 0:256],
                    xt_sb[:, dc, i, sub * 128 : (sub + 1) * 128],
                    w_sb[:, dc, 512:768],
                    start=(dc == 0),
                    stop=(dc == 7),
                )
            nc.vector.tensor_copy(
                out=vaug[:, :, nb, 0:64],
                in_=psv[:, 0:256].rearrange("p (h d) -> p h d", h=HPC),
            )

        # ---- Phase 2: attention + out-proj, per 512-token q-chunk ----
        for i in range(4):
            qsl = slice(i * 512, (i + 1) * 512)
            aoq = ao_pool.tile([128, 4, 256], BF16, tag="aoq")  # (q, qb, e_loc)
            for h in range(HPC):
                hs = slice((h % 2) * 64, (h % 2) * 64 + 64)
                ho = h // 2
                exps = exps_pool.tile([128, 16, 512], BF16, tag="exps")
                for kp in range(8):
                    ps2 = ps_sc.tile([128, 2, 512], F32, tag="sc")
                    for j in range(2):
                        kb = kp * 2 + j
                        nc.tensor.matmul(
                            ps2[:, j, :],
                            kT[hs, ho, kb * 128 : (kb + 1) * 128],
                            qT[hs, ho, qsl],
                            start=True,
                            stop=True,
                        )
                    nc.scalar.activation(
                        exps[:, kp * 2 : kp * 2 + 2, :], ps2[:], AF.Exp
                    )
                psu = ps_u.tile([65, 512], F32, tag="psu")
                for kb in range(16):
                    nc.tensor.matmul(
                        psu[:],
                        vaug[:, h, kb, :],
                        exps[:, kb, :],
                        start=(kb == 0),
                        stop=(kb == 15),
                    )
                ub = u_pool.tile([65, 512], BF16, tag="ub")
                nc.vector.tensor_copy(out=ub[:], in_=psu[:])
                pst = ps_t.tile([128, 8, 128], BF16, tag="pst")
                for qb in range(4):
                    nc.tensor.transpose(
                        pst[:, qb, 0:65], ub[:, qb * 128 : (qb + 1) * 128],
                        ident[0:65, 0:65],
                    )
                rec = rec_pool.tile([128, 4], F32, tag="rec")
                nc.vector.reciprocal(out=rec[:], in_=pst[:, 0:4, 64])
                for qb in range(4):
                    nc.vector.tensor_scalar_mul(
                        aoq[:, qb, h * 64 : (h + 1) * 64],
                        pst[:, qb, 0:64],
                        rec[:, qb : qb + 1],
                    )
            # transpose back (e on partitions) and out-project, per 128-row
            # block so the pipeline tail is one block deep
            pstT = ps_t.tile([128, 8, 128], BF16, tag="pst")
            for qb in range(4):
                aoT = aot_pool.tile([128, 2, 128], BF16, tag="aoT")
                for eo in range(2):
                    nc.tensor.transpose(
                        pstT[:, qb * 2 + eo, :],
                        aoq[:, qb, eo * 128 : (eo + 1) * 128],
                        ident[:],
                    )
                nc.vector.tensor_copy(
                    out=aoT[:], in_=pstT[:, qb * 2 : qb * 2 + 2, :]
                )
                for dc in range(2):
                    dsl = slice(dc * 512, (dc + 1) * 512)
                    psy = ps_big.tile([128, 512], F32, tag="big", name=f"y_ps{i}_{qb}_{dc}")
                    nc.tensor.matmul(
                        psy[:],
                        aoT[:, 0, :],
                        wo_sb[:, 0, dsl],
                        start=True,
                        stop=False,
                    )
                    nc.tensor.matmul(
                        psy[:],
                        aoT[:, 1, :],
                        wo_sb[:, 1, dsl],
                        start=False,
                        stop=True,
                    )
                    ysb = y_pool.tile([128, 512], BF16, tag="ysb")
                    nc.vector.tensor_copy(out=ysb[:], in_=psy[:])
                    nc.sync.dma_start(
                        out=y[i * 512 + qb * 128 : i * 512 + (qb + 1) * 128, dsl],
                        in_=ysb[:],
                    )


def make_in_maps(x, W_qkv, b_qkv, W_o):
    bf = ml_dtypes.bfloat16
    in_maps = []
    xt_b = []
    for b in range(B):
        x2 = np.asarray(x[b], dtype=np.float32)  # [2048, 1024]
        # [n, d] -> [d, n] -> [8, 128, n] -> [128, 8, n] -> chunk n by 512
        xt = x2.T.reshape(8, 128, N).transpose(1, 0, 2)
        xt = np.ascontiguousarray(
            xt.reshape(128, 8, 4, 512).transpose(2, 0, 1, 3)
        ).astype(bf)
        xt_b.append(xt)
    for c in range(NCORES):
        b, hg = c // 4, c % 4
        sl = slice(hg * 256, hg * 256 + 256)
        Wc = np.concatenate(
            [W_qkv[0:D][sl], W_qkv[D : 2 * D][sl], W_qkv[2 * D : 3 * D][sl]],
            axis=0,
        )  # [768, 1024]
        wc = np.ascontiguousarray(
            Wc.T.reshape(8, 128, 768).transpose(1, 0, 2)
        ).astype(bf)
        bc = np.concatenate(
            [b_qkv[0:D][sl] * 0.125, b_qkv[D : 2 * D][sl]]
        ).astype(np.float32)  # [512]
        bc = np.ascontiguousarray(bc.reshape(4, 128).T)  # [128, 4]
        woc = np.ascontiguousarray(
            W_o[:, sl].T.reshape(2, 128, 1024).transpose(1, 0, 2)
        ).astype(bf)  # [128, 2, 1024]
        in_maps.append({"xt": xt_b[b], "wqkv": wc, "bqkv": bc, "wo": woc})
    return in_maps


_NC_CACHE = {}


def get_nc():
    if "nc" not in _NC_CACHE:
        _NC_CACHE["nc"] = build_nc()
    return _NC_CACHE["nc"]


def run(inputs, trace=False, **kw):
    nc = get_nc()
    x = np.asarray(inputs["x"])
    W_qkv = np.asarray(inputs["W_qkv"], dtype=np.float32)
    b_qkv = np.asarray(inputs["b_qkv"], dtype=np.float32)
    W_o = np.asarray(inputs["W_o"], dtype=np.float32)
    b_o = np.asarray(inputs["b_o"], dtype=np.float32)
    in_maps = make_in_maps(x, W_qkv, b_qkv, W_o)
    res = run_bass_kernel_spmd(
        nc, in_maps, core_ids=list(range(NCORES)), trace=trace, **kw
    )
    parts = [np.asarray(m["y"], dtype=np.float32) for m in res.results]
    yb = []
    for b in range(B):
        yy = parts[4 * b]
        for g in range(1, 4):
            yy = yy + parts[4 * b + g]
        yb.append(yy)
    yout = np.stack(yb, axis=0)  # [B, N, D]
    # exact bias correction: v-bias flows through attention (rows sum to 1)
    # into out-proj; k-bias cancels in softmax; q-bias applied on device.
    corr = W_o @ b_qkv[2 * D : 3 * D] + b_o
    yout = yout + corr[None, None, :]
    return yout, res


def kernel(x, W_qkv, b_qkv, W_o, b_o):
    y, _ = run({"x": x, "W_qkv": W_qkv, "b_qkv": b_qkv, "W_o": W_o, "b_o": b_o})
    return y
